# revision 1
# baseline (speedup 1.0000x reference)
"""Trainium2 Bass kernel for nn_MihGNNEmbeddingTest3 (gnn_message_passing).

Reference math:
    H = mlp(A_s @ emb)          (mlp = 3 linear layers, no activations)
    out[e] = relu(|<H[src_e], H[dst_e]>| / (||H[src_e]|| ||H[dst_e]||))

Since the mlp is affine, fold it:  H = A_s @ (emb @ W_eff^T) + b_eff
(E2 = emb @ W_eff^T precomputed on host).  Per core: 1024 node rows of
H via 512 bf16 matmuls (4 pipelined m-groups; A shipped pre-transposed
partition-major so k-tiles land directly as lhsT), bias add streamed on
DVE, h rows stored to DRAM on the scalar HWDGE ring (so they never
queue behind the 16MB A-load stream on the sync ring), two chunked
AllGathers (first overlapped under the second half of the matmuls),
then per-side dma_gather row gathers (512 rows/call, int16 wrapped
indices) + dot/norm reductions split in halves so DVE/ACT math overlaps
the later gathers' Q7 descriptor generation.

Env toggles kept from tuning (defaults are the fast, validated path):
A_FP8=1 ships A as mean-shifted fp8 (bias-corrected, ~1e-3 rel err) via
SWDGE cast-DMA; NORM=1 enables the per-node normalize chain (HANGS the
device in this environment — do not enable); AG_MODE/GATHER_MODE select
fallback collective/gather implementations.
"""

import os
import sys

import numpy as np

try:
    import concourse.bass  # noqa: F401
except ImportError:  # pragma: no cover - grading env should have PYTHONPATH set
    for p in ("/opt/trn_rl_repo", "/root/.axon_site/_ro/trn_rl_repo"):
        if os.path.isdir(p) and p not in sys.path:
            sys.path.insert(0, p)

import ml_dtypes

N, D, B = 8192, 256, 8192
N_CORES = 8
ROWS = N // N_CORES     # A_s rows / nodes per core
EPC = B // N_CORES      # edges per core
KT = N // 128           # contraction tiles
MT = ROWS // 128        # output row m-tiles per core (8)
JT = EPC // 128         # edge blocks per core (8)
NG = 4                  # m-groups / AllGather chunks
MPG = MT // NG          # m-tiles per group (2)
GROWS = MPG * 128       # rows per group per core (256)

_CACHE = {}
LAST_RESULTS = None  # BassKernelResults of the most recent run (for test.py)


def _ind_gathers(nc, bass, hs_all, hd_all, h_full, sidx_sb, didx_sb):
    gb = int(os.environ.get("GATHER_BATCH", "1"))
    for j0 in range(0, JT, gb):
        j1 = min(j0 + gb, JT)
        nc.gpsimd.indirect_dma_start(
            out=hs_all[:, j0:j1, :],
            out_offset=None,
            in_=h_full[:],
            in_offset=bass.IndirectOffsetOnAxis(ap=sidx_sb[:, j0:j1], axis=0),
        )
        nc.gpsimd.indirect_dma_start(
            out=hd_all[:, j0:j1, :],
            out_offset=None,
            in_=h_full[:],
            in_offset=bass.IndirectOffsetOnAxis(ap=didx_sb[:, j0:j1], axis=0),
        )


def _build():
    import concourse.bacc as bacc
    import concourse.bass as bass
    import concourse.mybir as mybir
    import concourse.tile as tile

    fp32 = mybir.dt.float32
    bf16 = mybir.dt.bfloat16

    nc = bacc.Bacc(num_devices=N_CORES)
    fp8 = mybir.dt.float8e4
    a_fp8 = os.environ.get("A_FP8", "0") == "1"
    st_eng = nc.sync if os.environ.get("NOSCALARDMA") == "1" else nc.scalar
    use_norm = os.environ.get("NORM") == "1"
    # partition-major layouts: [p, k_tile, cols]; one DRAM param per m-group
    # so each group's data is one contiguous span.
    at_dt = mybir.dt.float8e4 if os.environ.get("A_FP8", "0") == "1" else bf16
    ats = [
        nc.declare_dram_parameter(f"at{g}", [128, KT, GROWS], at_dt, isOutput=False)
        for g in range(NG)
    ]
    e2 = nc.declare_dram_parameter("e2", [128, KT, D], bf16, isOutput=False)
    sidx16 = nc.declare_dram_parameter(
        "sidx16", [128, EPC // 16], mybir.dt.int16, isOutput=False)
    didx16 = nc.declare_dram_parameter(
        "didx16", [128, EPC // 16], mybir.dt.int16, isOutput=False)
    bias = nc.declare_dram_parameter("bias", [128, D], fp32, isOutput=False)
    scidx = nc.declare_dram_parameter("scidx", [128, MT], mybir.dt.int32, isOutput=False)
    hofs = nc.declare_dram_parameter("hofs", [1, 2], mybir.dt.int32, isOutput=False)
    sidx = nc.declare_dram_parameter("sidx", [128, JT], mybir.dt.int32, isOutput=False)
    didx = nc.declare_dram_parameter("didx", [128, JT], mybir.dt.int32, isOutput=False)
    out = nc.declare_dram_parameter("out", [128, JT], fp32, isOutput=True)

    with tile.TileContext(nc) as tc:
        with (
            tc.tile_pool(name="atp", bufs=1) as atp,
            tc.tile_pool(name="e2p", bufs=1) as e2p,
            tc.tile_pool(name="psum", bufs=MT, space="PSUM") as psum,
            tc.tile_pool(name="hwork", bufs=2) as hwork,
            tc.tile_pool(name="dram", bufs=1, space="DRAM") as dram,
            tc.tile_pool(name="const", bufs=1) as constp,
            tc.tile_pool(name="gat", bufs=1) as gat,
            tc.tile_pool(name="small", bufs=1) as small,
        ):
            ag_mode = os.environ.get("AG_MODE", "cc2")
            use_scatter = ag_mode == "scatter"
            h_shard = dram.tile([ROWS, D], bf16)
            # uneven chunks (m-tiles 0-1 / 2-3 / 4-7): the first small AG
            # starts right after group 0 so the CC pipeline opens ~14us
            # earlier; the last doorbell time is unchanged.
            CH_MT = [2, 4, 2] if os.environ.get("AG3", "1") == "1" else [4, 4]
            CH_OF = [sum(CH_MT[:i]) for i in range(len(CH_MT))]   # first m-tile
            CH_TRG = [o + n - 1 for o, n in zip(CH_OF, CH_MT)]    # last m-tile
            h_shg = [
                dram.tile([n * 128, D], bf16, name=f"h_shg{i}")
                for i, n in enumerate(CH_MT)
            ]
            h_space = "Local" if (ag_mode == "cc2" or os.environ.get("H_LOCAL") == "1") else "Shared"
            h_full = dram.tile([N, D], bf16, addr_space=h_space)
            bar_in = dram.tile([8, 16], fp32)
            bar_out = dram.tile([8 * N_CORES, 16], fp32, addr_space=h_space)

            # Small latency-critical loads FIRST so they never queue behind
            # the 16MB A stream on the sync HWDGE ring.
            bias_sb = constp.tile([128, D], fp32)
            nc.sync.dma_start(out=bias_sb[:], in_=bias[:])
            if use_scatter:
                nc.sync.dma_start(out=bar_in[:], in_=bias_sb[0:8, 0:16])
            use_dg = os.environ.get("GATHER_MODE", "dg") == "dg"
            sidx_sb = constp.tile([128, JT], mybir.dt.int32)
            didx_sb = constp.tile([128, JT], mybir.dt.int32)
            if not use_dg:
                nc.sync.dma_start(out=sidx_sb[:], in_=sidx[:])
                nc.sync.dma_start(out=didx_sb[:], in_=didx[:])
            scidx_sb = constp.tile([128, MT], mybir.dt.int32)
            hofs_sb = constp.tile([1, 2], mybir.dt.int32)
            sidx16_sb = constp.tile([128, EPC // 16], mybir.dt.int16)
            didx16_sb = constp.tile([128, EPC // 16], mybir.dt.int16)

            # Batched loads: few big DMAs. Small leading chunks so the first
            # matmuls start early; e2 and group-0 A interleave so k-tiles
            # arrive in lockstep.
            AT0_BOUNDS = [0, 2, 6, 14, 30, 64]
            E2_BOUNDS = [0, 2, 6, 14, 30, 64]
            ATN_BOUNDS = [0, 22, 43, 64]
            at_t = [[None] * KT for _ in range(NG)]  # [group][k] -> AP [128, GROWS]
            e2_t = [None] * KT

            def load_e2(lo, hi):
                ec = e2p.tile([128, hi - lo, D], bf16, name=f"e2c_{lo}", tag=f"e2c{lo}")
                nc.sync.dma_start(out=ec[:], in_=e2[:, lo:hi, :])
                for k in range(lo, hi):
                    e2_t[k] = ec[:, k - lo, :]

            def load_at(g, lo, hi):
                ac = atp.tile(
                    [128, hi - lo, GROWS], bf16,
                    name=f"atc_{g}_{lo}", tag=f"atc{g}_{lo}",
                )
                if a_fp8:
                    # SWDGE cast-DMA: fp8 in HBM -> bf16 in SBUF (halves HBM read)
                    nc.gpsimd.dma_start(out=ac[:], in_=ats[g][:, lo:hi, :])
                else:
                    nc.sync.dma_start(out=ac[:], in_=ats[g][:, lo:hi, :])
                for k in range(lo, hi):
                    at_t[g][k] = ac[:, k - lo, :]

            for ci in range(len(AT0_BOUNDS) - 1):
                load_e2(E2_BOUNDS[ci], E2_BOUNDS[ci + 1])
                load_at(0, AT0_BOUNDS[ci], AT0_BOUNDS[ci + 1])
                if ci == 0:
                    if use_dg:
                        nc.sync.dma_start(out=sidx16_sb[:], in_=sidx16[:])
                        nc.sync.dma_start(out=didx16_sb[:], in_=didx16[:])
                    if os.environ.get("AG_MODE", "cc2") == "scatter":
                        nc.sync.dma_start(out=scidx_sb[:], in_=scidx[:])
                        nc.sync.dma_start(out=hofs_sb[:], in_=hofs[:])
            for g in range(1, NG):
                for ci in range(len(ATN_BOUNDS) - 1):
                    load_at(g, ATN_BOUNDS[ci], ATN_BOUNDS[ci + 1])

            # tiny early sqrt: hoists the Sqrt ACT-table load out of the
            # edge-phase critical tail (Square alone would pick a table set
            # without Sqrt, forcing a 1.3us reload right before the output)
            warm = small.tile([128, 2], fp32, name="warm", tag="warm")
            nc.scalar.sqrt(warm[:, 0:1], bias_sb[:, 0:1])
            # node norms: ss/st/inv columns per m-tile
            ss = small.tile([128, MT], fp32, name="ss", tag="ss")
            st = small.tile([128, MT], fp32, name="st", tag="st")
            inv = small.tile([128, MT], fp32, name="inv", tag="inv")
            out_sb = constp.tile([128, JT], fp32)

            scatters = []
            hb_all = gat.tile([128, MT, D], bf16, name="hb_all", tag="hb_all")
            with nc.named_scope("matmul"):
                ps_t = [
                    psum.tile([128, D], fp32, name=f"ps_{m}", tag="ps")
                    for m in range(MT)
                ]
                ag_chunks = []
                for g in range(NG):
                    ms = range(g * MPG, (g + 1) * MPG)
                    # last group runs m-outer so m6's psum (and its store)
                    # completes ~11us before the final matmul, shortening the
                    # last AllGather's doorbell chain to m7's store alone
                    order = (
                        [(k, m) for m in ms for k in range(KT)]
                        if g == NG - 1 else
                        [(k, m) for k in range(KT) for m in ms]
                    )
                    for k, m in order:
                        lm = m - g * MPG
                        nc.tensor.matmul(
                            out=ps_t[m][:],
                            lhsT=at_t[g][k][:, lm * 128:(lm + 1) * 128],
                            rhs=e2_t[k],
                            start=(k == 0),
                            stop=(k == KT - 1),
                        )
                    with nc.named_scope(f"norm{g}"):
                        for m in ms:
                            if use_norm:
                                t = hwork.tile([128, D], fp32, name=f"t_{m}", tag="t")
                                nc.vector.tensor_tensor(
                                    out=t[:], in0=ps_t[m][:], in1=bias_sb[:],
                                    op=mybir.AluOpType.add,
                                )
                                sq = hwork.tile([128, D], fp32, name=f"sq_{m}", tag="sq")
                                if os.environ.get("NORM_IMPL", "dve") == "act":
                                    nc.scalar.activation(
                                        out=sq[:], in_=t[:],
                                        func=mybir.ActivationFunctionType.Square,
                                        accum_out=ss[:, m:m + 1],
                                    )
                                else:
                                    nc.vector.tensor_tensor_reduce(
                                        out=sq[:], in0=t[:], in1=t[:],
                                        scale=1.0, scalar=0.0,
                                        op0=mybir.AluOpType.mult,
                                        op1=mybir.AluOpType.add,
                                        accum_out=ss[:, m:m + 1],
                                    )
                                nc.scalar.sqrt(st[:, m:m + 1], ss[:, m:m + 1])
                                nc.vector.reciprocal(inv[:, m:m + 1], st[:, m:m + 1])
                                hb = hb_all[:, m, :]
                                if os.environ.get("NORM_IMPL", "dve") == "act":
                                    nc.scalar.activation(
                                        out=hb, in_=t[:],
                                        func=mybir.ActivationFunctionType.Copy,
                                        bias=0.0, scale=inv[:, m:m + 1],
                                    )
                                else:
                                    nc.vector.tensor_scalar(
                                        out=hb, in0=t[:],
                                        scalar1=inv[:, m:m + 1], scalar2=None,
                                        op0=mybir.AluOpType.mult,
                                    )
                            else:
                                hb = hb_all[:, m, :]
                                nc.vector.tensor_tensor(
                                    out=hb, in0=ps_t[m][:], in1=bias_sb[:],
                                    op=mybir.AluOpType.add,
                                )
                            if ag_mode == "cc2":
                                ch = max(
                                    i for i, o in enumerate(CH_OF) if m >= o
                                )
                                lm2 = m - CH_OF[ch]
                                # SWDGE store: its completion sem does not
                                # alias the sync-ring A-load lanes, so the
                                # AllGather doorbell fires as soon as the
                                # chunk is really ready (was +18us late)
                                h_st = (nc.gpsimd if os.environ.get(
                                    "H_ENG", "gpsimd") == "gpsimd" else st_eng)
                                h_st.dma_start(
                                    out=h_shg[ch][lm2 * 128:(lm2 + 1) * 128, :],
                                    in_=hb,
                                )

                    last_m = g * MPG + MPG - 1
                    if ag_mode == "cc2" and last_m in CH_TRG:
                        ch = CH_TRG.index(last_m)
                        base = CH_OF[ch] * 128 * N_CORES
                        size = CH_MT[ch] * 128 * N_CORES
                        with nc.named_scope(f"ag{ch}"):
                            nc.gpsimd.collective_compute(
                                "AllGather",
                                mybir.AluOpType.bypass,
                                replica_groups=[list(range(N_CORES))],
                                ins=[h_shg[ch][:]],
                                outs=[h_full[base:base + size, :]],
                            )
                with nc.named_scope("allgather"):
                    if ag_mode == "cc2":
                        pass
                    elif use_scatter:
                        ofs = nc.sync.value_load(
                            hofs_sb[0:1, 0:1], min_val=0, max_val=(N_CORES - 1) * ROWS
                        )
                        hf = h_full[:]
                        dyn_out = bass.AP(
                            hf.tensor,
                            ofs * D,
                            [[D, 128], [128 * D, MT], [1, D]],
                        )
                        hw = nc.sync.dma_start(out=dyn_out, in_=hb_all[:])
                        if os.environ.get("NOBAR") == "1":
                            bar_cc = hw
                        else:
                            bar_cc = nc.gpsimd.collective_compute(
                                "AllReduce",
                                mybir.AluOpType.add,
                                replica_groups=[list(range(N_CORES))],
                                ins=[bar_in[:].opt()],
                                outs=[bar_in[:].opt()],
                            )
                            bass._add_dep_helper(
                                bar_cc.ins, hw.ins, sync=True,
                                reason="barrier waits for h write",
                            )
                    else:
                        for m in range(MT):
                            st_eng.dma_start(
                                out=h_shard[m * 128:(m + 1) * 128, :],
                                in_=hb_all[:, m, :],
                            )
                        bar_cc = nc.gpsimd.collective_compute(
                            "AllGather",
                            mybir.AluOpType.bypass,
                            replica_groups=[list(range(N_CORES))],
                            ins=[h_shard[:]],
                            outs=[h_full[:]],
                        )

            with nc.named_scope("edges"):
                hs_all = gat.tile([128, JT, D], bf16, name="hs_all", tag="hs_all")
                hd_all = gat.tile([128, JT, D], bf16, name="hd_all", tag="hd_all")
                QN = int(os.environ.get("GATHER_SPLIT", "4"))
                HEPC = EPC // QN
                HJT = JT // QN
                gs = []
                if os.environ.get("GATHER_MODE", "dg") == "dg":
                    for h in range(QN):
                        js = slice(h * HJT, (h + 1) * HJT)
                        cs = slice(h * (HEPC // 16), (h + 1) * (HEPC // 16))
                        gs.append(nc.gpsimd.dma_gather(
                            hs_all[:, js, :], h_full[:], sidx16_sb[:, cs],
                            HEPC, HEPC, D))
                        gs.append(nc.gpsimd.dma_gather(
                            hd_all[:, js, :], h_full[:], didx16_sb[:, cs],
                            HEPC, HEPC, D))
                    if use_scatter:
                        for gg in gs:
                            bass._add_dep_helper(
                                gg.ins, bar_cc.ins, sync=True,
                                reason="gathers wait for cross-core barrier")
                else:
                    _ind_gathers(nc, bass, hs_all, hd_all, h_full, sidx_sb, didx_sb)
                dot = small.tile([128, JT], fp32, name="dot", tag="dot")

                if use_norm and os.environ.get("EDGE_IMPL", "new") == "new":
                    for j in range(JT):
                        prod = hwork.tile([128, D], fp32, name=f"prod_{j}", tag="prod")
                        nc.vector.tensor_tensor_reduce(
                            out=prod[:],
                            in0=hs_all[:, j, :],
                            in1=hd_all[:, j, :],
                            scale=1.0,
                            scalar=0.0,
                            op0=mybir.AluOpType.mult,
                            op1=mybir.AluOpType.add,
                            accum_out=dot[:, j:j + 1],
                        )
                    nc.scalar.activation(
                        out=out_sb[:], in_=dot[:],
                        func=mybir.ActivationFunctionType.Abs,
                    )
                else:
                    ns = small.tile([128, JT], fp32, name="ns", tag="ns")
                    nd = small.tile([128, JT], fp32, name="nd", tag="nd")
                    prod = gat.tile([128, JT, D], fp32, name="prod", tag="prod")
                    sq_s = gat.tile([128, JT, D], fp32, name="sq_s", tag="sq_s")
                    sq_d = gat.tile([128, JT, D], fp32, name="sq_d", tag="sq_d")
                    for h in range(QN):
                        js = slice(h * HJT, (h + 1) * HJT)
                        nc.vector.tensor_tensor(
                            out=prod[:, js, :], in0=hs_all[:, js, :],
                            in1=hd_all[:, js, :],
                            op=mybir.AluOpType.mult,
                        )
                        nc.vector.tensor_reduce(
                            out=dot[:, js], in_=prod[:, js, :],
                            axis=mybir.AxisListType.X,
                            op=mybir.AluOpType.add,
                        )
                        nc.scalar.square(sq_s[:, js, :], hs_all[:, js, :])
                        nc.scalar.square(sq_d[:, js, :], hd_all[:, js, :])
                        nc.vector.tensor_reduce(
                            out=ns[:, js], in_=sq_s[:, js, :],
                            axis=mybir.AxisListType.X,
                            op=mybir.AluOpType.add,
                        )
                        nc.vector.tensor_reduce(
                            out=nd[:, js], in_=sq_d[:, js, :],
                            axis=mybir.AxisListType.X,
                            op=mybir.AluOpType.add,
                        )
                        nsnd = small.tile([128, JT], fp32, name="nsnd", tag="nsnd")
                        nc.vector.tensor_tensor(
                            out=nsnd[:, js], in0=ns[:, js], in1=nd[:, js],
                            op=mybir.AluOpType.mult,
                        )
                        stq = small.tile([128, JT], fp32, name="stq", tag="stq")
                        nc.scalar.sqrt(stq[:, js], nsnd[:, js])
                        invq = small.tile([128, JT], fp32, name="invq", tag="invq")
                        nc.vector.reciprocal(invq[:, js], stq[:, js])
                        ad = small.tile([128, JT], fp32, name="ad", tag="ad")
                        nc.vector.tensor_scalar(
                            out=ad[:, js].bitcast(mybir.dt.uint32),
                            in0=dot[:, js].bitcast(mybir.dt.uint32),
                            scalar1=0x7FFFFFFF, scalar2=None,
                            op0=mybir.AluOpType.bitwise_and,
                        )
                        nc.vector.tensor_tensor(
                            out=out_sb[:, js], in0=ad[:, js], in1=invq[:, js],
                            op=mybir.AluOpType.mult,
                        )

            st_eng.dma_start(out=out[:], in_=out_sb[:])

    nc.compile()
    return nc


def _get_nc():
    if "nc" not in _CACHE:
        _CACHE["nc"] = _build()
    return _CACHE["nc"]


def _remap(n):
    # node id -> h_full row. cc2 mode: two chunked AllGathers; chunk g holds
    # rows [g*512, (g+1)*512) of every core shard, concatenated rank-major.
    if os.environ.get("AG_MODE", "cc2") != "cc2":
        return n
    ch_mt = [2, 4, 2] if os.environ.get("AG3", "1") == "1" else [4, 4]
    ch_of = np.array([sum(ch_mt[:i]) for i in range(len(ch_mt))]) * 128
    ch_rows = np.array(ch_mt) * 128
    o = n // ROWS
    l = n % ROWS
    g = np.searchsorted(ch_of, l, side="right") - 1
    return ch_of[g] * N_CORES + o * ch_rows[g] + (l - ch_of[g])


def kernel(edges, A_s, emb, Ws, bs):
    global LAST_RESULTS
    from concourse.bass_utils import run_bass_kernel_spmd

    bf16 = ml_dtypes.bfloat16
    A = np.asarray(A_s, dtype=np.float32)
    E = np.asarray(emb, dtype=np.float32)
    W = np.asarray(Ws, dtype=np.float32)
    b = np.asarray(bs, dtype=np.float32)
    ed = np.asarray(edges)

    a_fp8 = os.environ.get("A_FP8", "0") == "1"
    f8 = ml_dtypes.float8_e4m3fn
    M = W[0].T @ W[1].T @ W[2].T                      # [D, D]
    E2f = (E @ M).astype(bf16)                        # [N, D] as used on device
    # partition-major: [128(p), KT(t), D] with row t*128+p at [p, t, :]
    E2 = np.ascontiguousarray(E2f.reshape(KT, 128, D).transpose(1, 0, 2))
    b_eff = (b[0] @ W[1].T + b[1]) @ W[2].T + b[2]    # [D]
    if a_fp8:
        # A shipped as fp8(A - 0.5); fold the +0.5 row-sum term into the bias
        b_eff = b_eff + 0.5 * E2f.astype(np.float32).sum(0)
    bias_rep = np.ascontiguousarray(
        np.broadcast_to(b_eff.astype(np.float32), (128, D))
    )

    in_maps = []
    for c in range(N_CORES):
        m = {"e2": E2, "bias": bias_rep}
        for g in range(NG):
            r0 = c * ROWS + g * GROWS
            blk = A[r0:r0 + GROWS, :].T                   # [N, GROWS]
            blk = (blk - 0.5).astype(f8) if a_fp8 else blk.astype(bf16)
            m[f"at{g}"] = np.ascontiguousarray(
                blk.reshape(KT, 128, GROWS).transpose(1, 0, 2)
            )
        e = ed[c * EPC:(c + 1) * EPC].astype(np.int64)
        m["sidx"] = np.ascontiguousarray(
            _remap(e[:, 0]).astype(np.int32).reshape(JT, 128).T
        )
        dsrc = e[:, 0] if os.environ.get("PROBE_DD_EQ_SS") == "1" else e[:, 1]
        m["didx"] = np.ascontiguousarray(
            _remap(dsrc).astype(np.int32).reshape(JT, 128).T
        )

        QN = int(os.environ.get("GATHER_SPLIT", "4"))

        def wrap16(flat):
            # QN independent gathers: wrap each EPC/QN-index chunk separately
            def w(f):
                buf = f.astype(np.int16).reshape(-1, 16).T
                return np.tile(buf, (8, 1))
            step = EPC // QN
            return np.ascontiguousarray(
                np.hstack([w(flat[q * step:(q + 1) * step]) for q in range(QN)]))

        m["hofs"] = np.array([[c * ROWS, 0]], dtype=np.int32)
        m["scidx"] = np.ascontiguousarray(
            (c * ROWS + np.arange(MT)[None, :] * 128
             + np.arange(128)[:, None]).astype(np.int32))
        m["sidx16"] = wrap16(_remap(e[:, 0]))
        m["didx16"] = wrap16(_remap(dsrc))
        in_maps.append(m)

    nc = _get_nc()
    kw = {}
    if os.environ.get("KERNEL_TRACE_KW"):
        import json
        kw = json.loads(os.environ["KERNEL_TRACE_KW"])
    res = run_bass_kernel_spmd(nc, in_maps, list(range(N_CORES)), **kw)
    LAST_RESULTS = res

    out = np.concatenate(
        [np.ascontiguousarray(res.results[c]["out"].T).reshape(-1) for c in range(N_CORES)]
    )
    return np.maximum(out, 0.0).astype(np.float32)

